# revision 3
# baseline (speedup 1.0000x reference)
"""Multi-head attention on 8 Trainium2 NeuronCores (tensor-parallel over heads).

B=4, S=2048, D=1024, H=16 heads of DK=64. Each core owns 2 heads (a
128-channel slice of the QKV projections). Per core:
  xT   = transpose(x[b])                          (PE transposes)
  QT   = (Wq_c)^T x^T + bq_c      [128, S]        (K on partitions)
  KT   = (Wk_c)^T x^T + bk_c      [128, S]
  V    = x Wv_c + bv_c            [S, 128] stored per-head with ones col
  per head h, per q-tile: accumulate over k-chunks:
    scT  = K Q^T / 8   -> exp     [k=128, q=512]
    av  += V_aug^T expT           [65, 512] rows 0-63 ctx, row 64 = sumexp
  ctxT = av[0:64] * (1/av[64])    (bcast via DRAM bounce)
  out[b] partial = ctx^T Wo_c     [S, D]  (host sums partials + bo)
"""

import numpy as np

B, S, D, H, DK = 4, 2048, 1024, 16, 64
NCORES = 8
CS = D // NCORES  # 128 channels (2 heads) per core
NSB = S // 128    # 16 s-blocks
NST = S // 512    # 4 s-tiles
NDC = D // 128    # 8 d-chunks

TRACE = False
LAST_RESULTS = None
_CACHE = {}


def _build():
    import concourse.bass as bass  # noqa: F401
    import concourse.mybir as mybir
    import concourse.tile as tile
    from concourse import bacc
    from concourse.masks import make_identity

    fp32 = mybir.dt.float32
    AF = mybir.ActivationFunctionType

    nc = bacc.Bacc(None, target_bir_lowering=False)
    x_d = nc.declare_dram_parameter("x", [B, S, D], fp32, isOutput=False)
    wq_d = nc.declare_dram_parameter("wq", [D, CS], fp32, isOutput=False)
    wk_d = nc.declare_dram_parameter("wk", [D, CS], fp32, isOutput=False)
    wv_d = nc.declare_dram_parameter("wv", [D, CS], fp32, isOutput=False)
    wo_d = nc.declare_dram_parameter("wo", [CS, D], fp32, isOutput=False)
    bq_d = nc.declare_dram_parameter("bq", [CS], fp32, isOutput=False)
    bk_d = nc.declare_dram_parameter("bk", [CS], fp32, isOutput=False)
    bv_d = nc.declare_dram_parameter("bv", [CS], fp32, isOutput=False)
    out_d = nc.declare_dram_parameter("out", [B, S, D], fp32, isOutput=True)

    with tile.TileContext(nc) as tc:
        with (
            tc.tile_pool(name="consts", bufs=1) as consts,
            tc.tile_pool(name="xt", bufs=1) as xt_pool,
            tc.tile_pool(name="xload", bufs=3) as xload,
            tc.tile_pool(name="qk", bufs=1) as qk_pool,
            tc.tile_pool(name="vp", bufs=1) as v_pool,
            tc.tile_pool(name="exp", bufs=3) as exp_pool,
            tc.tile_pool(name="ctx", bufs=1) as ctx_pool,
            tc.tile_pool(name="rec", bufs=2) as rec_pool,
            tc.tile_pool(name="rb", bufs=2) as rb_pool,
            tc.tile_pool(name="outp", bufs=3) as out_pool,
            tc.tile_pool(name="drp", bufs=4, space="DRAM") as dram_pool,
            tc.tile_pool(name="ps", bufs=1, space="PSUM") as ps,
        ):
            # ---- constants ----
            wq_t = consts.tile([128, NDC, CS], fp32, tag="wq")
            nc.sync.dma_start(wq_t[:], wq_d[:].rearrange("(c p) m -> p c m", p=128))
            wk_t = consts.tile([128, NDC, CS], fp32, tag="wk")
            nc.sync.dma_start(wk_t[:], wk_d[:].rearrange("(c p) m -> p c m", p=128))
            wv_t = consts.tile([128, NDC, CS], fp32, tag="wv")
            nc.sync.dma_start(wv_t[:], wv_d[:].rearrange("(c p) m -> p c m", p=128))
            wo_t = consts.tile([128, D], fp32, tag="wo")
            nc.sync.dma_start(wo_t[:], wo_d[:])
            bq_t = consts.tile([128, 1], fp32, tag="bq")
            nc.sync.dma_start(bq_t[:], bq_d[:].rearrange("(p o) -> p o", o=1))
            bk_t = consts.tile([128, 1], fp32, tag="bk")
            nc.sync.dma_start(bk_t[:], bk_d[:].rearrange("(p o) -> p o", o=1))
            bv_b = consts.tile([128, CS], fp32, tag="bvb")
            nc.sync.dma_start(
                bv_b[:],
                bv_d[:].rearrange("(o f) -> o f", o=1).partition_broadcast(128),
            )
            ident = consts.tile([128, 128], fp32, tag="ident")
            make_identity(nc, ident[:])

            for b in range(B):
                # ---- Phase A: x transpose ----
                xT = xt_pool.tile([128, NDC, S], fp32, tag="xT")
                for sb in range(NSB):
                    xl = xload.tile([128, D], fp32, tag="xl")
                    nc.sync.dma_start(xl[:], x_d[b, sb * 128 : (sb + 1) * 128, :])
                    for cch in range(NDC):
                        pt = ps.tile([128, 128], fp32, tag="mm128")
                        nc.tensor.transpose(
                            pt[:], xl[:, cch * 128 : (cch + 1) * 128], ident[:]
                        )
                        nc.vector.tensor_copy(
                            xT[:, cch, sb * 128 : (sb + 1) * 128], pt[:]
                        )

                # ---- Phase A2: QT / KT projections ----
                QT = qk_pool.tile([128, S], fp32, tag="QT")
                KT = qk_pool.tile([128, S], fp32, tag="KT")
                for st in range(NST):
                    sl = slice(st * 512, (st + 1) * 512)
                    pq = ps.tile([128, 512], fp32, tag="mm512")
                    for cch in range(NDC):
                        nc.tensor.matmul(
                            pq[:],
                            wq_t[:, cch, :],
                            xT[:, cch, sl],
                            start=(cch == 0),
                            stop=(cch == NDC - 1),
                        )
                    nc.vector.tensor_scalar_add(QT[:, sl], pq[:], bq_t[:])
                    pk = ps.tile([128, 512], fp32, tag="mm512")
                    for cch in range(NDC):
                        nc.tensor.matmul(
                            pk[:],
                            wk_t[:, cch, :],
                            xT[:, cch, sl],
                            start=(cch == 0),
                            stop=(cch == NDC - 1),
                        )
                    nc.vector.tensor_scalar_add(KT[:, sl], pk[:], bk_t[:])

                # ---- Phase A3: V projection (per-head tiles with ones col) ----
                v0 = v_pool.tile([128, NSB, 65], fp32, tag="v0")
                v1 = v_pool.tile([128, NSB, 65], fp32, tag="v1")
                nc.gpsimd.memset(v0[:, :, 64:65], 1.0)
                nc.gpsimd.memset(v1[:, :, 64:65], 1.0)
                for sb in range(NSB):
                    pv = ps.tile([128, 128], fp32, tag="mm128")
                    for cch in range(NDC):
                        nc.tensor.matmul(
                            pv[:],
                            xT[:, cch, sb * 128 : (sb + 1) * 128],
                            wv_t[:, cch, :],
                            start=(cch == 0),
                            stop=(cch == NDC - 1),
                        )
                    nc.vector.tensor_add(v0[:, sb, 0:64], pv[:, 0:64], bv_b[:, 0:64])
                    nc.vector.tensor_add(v1[:, sb, 0:64], pv[:, 64:128], bv_b[:, 64:128])

                # ---- Phase B: attention per head ----
                ctxT = ctx_pool.tile([128, S], fp32, tag="ctxT")
                for h in (0, 1):
                    hoff = 64 * h
                    vh = v0 if h == 0 else v1
                    for qt in range(NST):
                        qsl = slice(qt * 512, (qt + 1) * 512)
                        av = ps.tile([65, 512], fp32, tag="av")
                        for kc in range(NSB):
                            ksl = slice(kc * 128, (kc + 1) * 128)
                            sc = ps.tile([128, 512], fp32, tag="mm512")
                            nc.tensor.matmul(
                                sc[:],
                                KT[hoff : hoff + 64, ksl],
                                QT[hoff : hoff + 64, qsl],
                                start=True,
                                stop=True,
                            )
                            ex = exp_pool.tile([128, 512], fp32, tag="ex")
                            nc.scalar.activation(ex[:], sc[:], AF.Exp, scale=0.125)
                            nc.tensor.matmul(
                                av[:],
                                vh[:, kc, :],
                                ex[:],
                                start=(kc == 0),
                                stop=(kc == NSB - 1),
                                skip_group_check=True,
                            )
                        # normalize: ctxT[hoff:hoff+64, qsl] = av[0:64] / av[64]
                        rec = rec_pool.tile([65, 512], fp32, tag="rec")
                        nc.vector.reciprocal(rec[64:65, :], av[64:65, :])
                        dr = dram_pool.tile([1, 512], fp32, tag="dr")
                        nc.sync.dma_start(dr[:], rec[64:65, :])
                        rb = rb_pool.tile([64, 512], fp32, tag="rb")
                        nc.sync.dma_start(rb[:], dr[:].partition_broadcast(64))
                        nc.vector.tensor_mul(
                            ctxT[hoff : hoff + 64, qsl], av[0:64, :], rb[:]
                        )

                # ---- Phase C: output projection ----
                for sb in range(NSB):
                    ssl = slice(sb * 128, (sb + 1) * 128)
                    ot = out_pool.tile([128, D], fp32, tag="ot")
                    for half in range(2):
                        osl = slice(half * 512, (half + 1) * 512)
                        po = ps.tile([128, 512], fp32, tag="mm512")
                        nc.tensor.matmul(
                            po[:], ctxT[:, ssl], wo_t[:, osl], start=True, stop=True
                        )
                        nc.vector.tensor_copy(ot[:, osl], po[:])
                    nc.sync.dma_start(out_d[b, ssl, :], ot[:])

    nc.compile()
    return nc


def _get_nc():
    if "nc" not in _CACHE:
        _CACHE["nc"] = _build()
    return _CACHE["nc"]


def kernel(**inputs):
    global LAST_RESULTS
    from concourse.bass_utils import run_bass_kernel_spmd

    x = np.ascontiguousarray(inputs["x"], dtype=np.float32)
    Wq = np.asarray(inputs["Wq"], dtype=np.float32)
    Wk = np.asarray(inputs["Wk"], dtype=np.float32)
    Wv = np.asarray(inputs["Wv"], dtype=np.float32)
    Wo = np.asarray(inputs["Wo"], dtype=np.float32)
    bq = np.asarray(inputs["bq"], dtype=np.float32)
    bk = np.asarray(inputs["bk"], dtype=np.float32)
    bv = np.asarray(inputs["bv"], dtype=np.float32)
    bo = np.asarray(inputs["bo"], dtype=np.float32)

    nc = _get_nc()
    in_maps = []
    for c in range(NCORES):
        cs = slice(CS * c, CS * (c + 1))
        in_maps.append(
            {
                "x": x,
                "wq": np.ascontiguousarray(Wq[:, cs]),
                "wk": np.ascontiguousarray(Wk[:, cs]),
                "wv": np.ascontiguousarray(Wv[:, cs]),
                "wo": np.ascontiguousarray(Wo[cs, :]),
                "bq": np.ascontiguousarray(bq[cs]),
                "bk": np.ascontiguousarray(bk[cs]),
                "bv": np.ascontiguousarray(bv[cs]),
            }
        )
    res = run_bass_kernel_spmd(
        nc, in_maps, core_ids=list(range(NCORES)), trace=TRACE
    )
    LAST_RESULTS = res
    acc = np.zeros((B, S, D), dtype=np.float64)
    for c in range(NCORES):
        acc += res.results[c]["out"]
    acc += bo
    return acc.astype(np.float32)
